# revision 8
# baseline (speedup 1.0000x reference)
"""Fused multi-head-attention-block kernel for 8 Trainium2 NeuronCores.

Reference computation (B=4, S=2048, D=1024):
    qp  = q @ Wq + bq
    k   = qp @ Wk
    v   = qp @ Wv + bv
    qk  = einsum('bsd,btd->bst', qp, k) * (D//16)**-0.25 + mask
    out = softmax(qk) @ v @ Wo + bo

Sharding: each core owns one (batch, half-of-queries) pair.  The two cores
of a batch split the projection phases (each computes qpT/kT/v for its own
1024 query rows only), then exchange kT and v via pair-wise AllGathers.
Attention + output projection run on each core's own rows.

All activations are kept transposed ([dim, seq]) so every matmul's
contraction dim lands on SBUF partitions with zero on-chip transposes:
    qpT = Wq^T @ qT          kT = Wk^T @ qpT        v = qpT^T @ Wv
    qkT[t,s] = kT^T@qpT      wvT = v^T @ E          out = wvT^T @ Wo
Softmax runs over the PARTITION dim of qkT: exp on ScalarE (scale fused,
no max-subtraction -- logits are bounded and exp(-1e9) -> 0 exactly on the
ACT LUT), multiplicative mask tiles M = exp(mask) from the host (handles
causal and arbitrary additive masks uniformly), denominator via a
ones-vector matmul accumulated across key tiles, division folded into the
PSUM eviction of the PV matmul.

Key-tile order after the gathers is rows [0:512 | 512:1024 | 1536:2048 |
1024:1536] (AllGather #1 carries both cores' chunk-A rows, #2 the chunk-B
rows).  With a causal mask, chunk A (query positions 0:512) only needs the
first 8 key tiles (= rows 0:1024 cover every core's chunk-A queries), so
the per-core loop structure is {8,16} key tiles -- identical on all cores
(SPMD), 25% fewer attention matmuls.  A host-side query split per half
makes this work: h=0 owns rows {0:512, 1536:2048}, h=1 owns rows
{512:1024, 1024:1536}; masks are data, so any mask is supported (dense
fallback {16,16}).

Matmuls run in float32r (TF32-like, 4x fp32 throughput, ~2e-4 rel err).
"""

import os
import sys
from contextlib import ExitStack

for _p in ("/opt/trn_rl_repo",):
    if _p not in sys.path and os.path.isdir(_p):
        sys.path.append(_p)

import numpy as np

import concourse.bass as bass
import concourse.mybir as mybir
import concourse.tile as tile
from concourse import bacc
from concourse.bass_utils import run_bass_kernel_spmd

B, S, D, N_HEAD = 4, 2048, 1024, 16
P = 128                      # SBUF partitions
NCORES = 8
SQC = 512                    # query-chunk width (free dim of every matmul)
DK = D // P                  # 8 contraction tiles
NKT = S // P                 # 16 key tiles
SCALE = float((D // N_HEAD) ** (-0.25))
F32 = mybir.dt.float32
F32R = mybir.dt.float32r

_PROGRAM_CACHE = {}


def _build_program(kt_depths):
    """Emit the SPMD Bass program.  kt_depths = (ktA, ktB): number of
    128-row key tiles processed for query chunk A (positions 0:512) and
    chunk B (positions 1536:2048)."""
    nc = bacc.Bacc("TRN2", target_bir_lowering=False, debug=False,
                   num_devices=NCORES)

    qT = nc.dram_tensor("qT", [D, 2 * SQC], F32R, kind="ExternalInput").ap()
    M = nc.dram_tensor("M", [sum(kt_depths), P, SQC], F32,
                       kind="ExternalInput").ap()
    wq = nc.dram_tensor("Wq", [D, D], F32R, kind="ExternalInput").ap()
    wk = nc.dram_tensor("Wk", [D, D], F32R, kind="ExternalInput").ap()
    wv = nc.dram_tensor("Wv", [D, D], F32R, kind="ExternalInput").ap()
    wo = nc.dram_tensor("Wo", [D, D], F32R, kind="ExternalInput").ap()
    bq = nc.dram_tensor("bq", [D], F32, kind="ExternalInput").ap()
    bv = nc.dram_tensor("bv", [D], F32, kind="ExternalInput").ap()
    bo = nc.dram_tensor("bo", [D], F32, kind="ExternalInput").ap()
    out = nc.dram_tensor("out", [2 * SQC, D], F32, kind="ExternalOutput").ap()

    groups = [[0, 1], [2, 3], [4, 5], [6, 7]]

    with tile.TileContext(nc) as tc, ExitStack() as stack:
        const = stack.enter_context(tc.tile_pool(name="const", bufs=1))
        ones_f = const.tile([P, 1], F32)
        nc.gpsimd.memset(ones_f[:], 1.0)
        ones_r = const.tile([P, 1], F32R)
        nc.vector.tensor_copy(ones_r[:], ones_f[:])
        bq_sb = const.tile([P, DK], F32)
        nc.sync.dma_start(out=bq_sb[:], in_=bq.rearrange("(a p) -> p a", p=P))
        bv_row = const.tile([1, D], F32)
        nc.sync.dma_start(out=bv_row[:], in_=bv.unsqueeze(0))
        bo_row = const.tile([1, D], F32)
        nc.sync.dma_start(out=bo_row[:], in_=bo.unsqueeze(0))
        bv_bc = const.tile([P, D], F32)
        nc.gpsimd.partition_broadcast(bv_bc[:], bv_row[:])
        bo_bc = const.tile([P, D], F32)
        nc.gpsimd.partition_broadcast(bo_bc[:], bo_row[:])

        # persistent activation storage
        qpa_p = stack.enter_context(tc.tile_pool(name="qpa", bufs=1))
        qpb_p = stack.enter_context(tc.tile_pool(name="qpb", bufs=1))
        v_p = stack.enter_context(tc.tile_pool(name="vp", bufs=1))
        qpa = [qpa_p.tile([P, SQC], F32R, name=f"qpa{d}") for d in range(DK)]
        qpb = [qpb_p.tile([P, SQC], F32R, name=f"qpb{d}") for d in range(DK)]
        vt = [v_p.tile([P, D], F32R, name=f"v{t}") for t in range(NKT)]

        psum_acc = stack.enter_context(
            tc.tile_pool(name="ps_acc", bufs=2, space="PSUM"))
        psum_den = stack.enter_context(
            tc.tile_pool(name="ps_den", bufs=1, space="PSUM"))
        psum_pv = stack.enter_context(
            tc.tile_pool(name="ps_pv", bufs=1, space="PSUM"))

        dram = stack.enter_context(tc.tile_pool(name="dram", bufs=1,
                                                space="DRAM"))
        # kT contributions/gathers in [kt, dk, 128, 128] tile layout
        kc = [dram.tile([NKT // 4, DK, P, P], F32R, name=f"kc{c}")
              for c in range(2)]
        kg = [dram.tile([NKT // 2, DK, P, P], F32R, name=f"kg{c}")
              for c in range(2)]
        vc = [dram.tile([NKT // 4, P, D], F32R, name=f"vc{c}")
              for c in range(2)]
        vg = [dram.tile([NKT // 2, P, D], F32R, name=f"vg{c}")
              for c in range(2)]

        # ---- Phases A-C: projections for this core's 1024 rows ----------
        with tc.tile_pool(name="wpool", bufs=9) as wp, \
             tc.tile_pool(name="qtpool", bufs=16) as qtp, \
             tc.tile_pool(name="kev", bufs=3) as kev, \
             tc.tile_pool(name="vev", bufs=3) as vev:

            w_q = [wp.tile([P, D], F32R, name=f"wq{d}", tag="w")
                   for d in range(DK)]
            qts = [[None] * DK for _ in range(2)]
            # interleave weight/activation loads so the first matmul's
            # inputs arrive early
            for dk in range(DK):
                nc.sync.dma_start(out=w_q[dk][:],
                                  in_=wq[dk * P:(dk + 1) * P, :])
                for ch in range(2):
                    t = qtp.tile([P, SQC], F32R, name=f"qt{ch}_{dk}",
                                 tag="qt")
                    nc.sync.dma_start(
                        out=t[:],
                        in_=qT[dk * P:(dk + 1) * P, ch * SQC:(ch + 1) * SQC])
                    qts[ch][dk] = t

            # Phase A: qpT for my two query chunks
            for ch, qp_dst in ((0, qpa), (1, qpb)):
                for do in range(DK):
                    acc = psum_acc.tile([P, SQC], F32, tag="acc")
                    for dk in range(DK):
                        nc.tensor.matmul(
                            out=acc[:], lhsT=w_q[dk][:, do * P:(do + 1) * P],
                            rhs=qts[ch][dk][:], start=(dk == 0),
                            stop=(dk == DK - 1))
                    nc.vector.tensor_tensor(
                        qp_dst[do][:], acc[:],
                        bq_sb[:, do:do + 1].to_broadcast([P, SQC]),
                        mybir.AluOpType.add)

            # Phase B: kT for my rows -> contribution buffers -> AllGather
            w_k = [wp.tile([P, D], F32R, name=f"wk{d}", tag="w")
                   for d in range(DK)]
            for d in range(DK):
                nc.sync.dma_start(out=w_k[d][:], in_=wk[d * P:(d + 1) * P, :])
            for ch, qp_src in ((0, qpa), (1, qpb)):
                for do in range(DK):
                    acc = psum_acc.tile([P, SQC], F32, tag="acc")
                    for dk in range(DK):
                        nc.tensor.matmul(
                            out=acc[:], lhsT=w_k[dk][:, do * P:(do + 1) * P],
                            rhs=qp_src[dk][:], start=(dk == 0),
                            stop=(dk == DK - 1))
                    kt_sb = kev.tile([P, SQC], F32R, name=f"kev{ch}_{do}",
                                     tag="kev")
                    nc.vector.tensor_copy(kt_sb[:], acc[:])
                    nc.sync.dma_start(
                        out=kc[ch][0:4, do].transpose([1, 0, 2]),
                        in_=kt_sb[:].rearrange("p (a f) -> p a f", a=4))

            # Phase C: v for my rows -> contribution buffers -> AllGather
            w_v = [wp.tile([P, D], F32R, name=f"wv{d}", tag="w")
                   for d in range(DK)]
            for d in range(DK):
                nc.sync.dma_start(out=w_v[d][:], in_=wv[d * P:(d + 1) * P, :])
            for ch, qp_src in ((0, qpa), (1, qpb)):
                for st in range(4):
                    for nch in range(2):
                        acc = psum_acc.tile([P, SQC], F32, tag="acc")
                        for dk in range(DK):
                            nc.tensor.matmul(
                                out=acc[:],
                                lhsT=qp_src[dk][:, st * P:(st + 1) * P],
                                rhs=w_v[dk][:, nch * SQC:(nch + 1) * SQC],
                                start=(dk == 0), stop=(dk == DK - 1))
                        v_sb = vev.tile([P, SQC], F32R,
                                        name=f"vev{ch}_{st}_{nch}", tag="vev")
                        nc.vector.tensor_tensor(
                            v_sb[:], acc[:],
                            bv_bc[:, nch * SQC:(nch + 1) * SQC],
                            mybir.AluOpType.add)
                        nc.sync.dma_start(
                            out=vc[ch][st, :, nch * SQC:(nch + 1) * SQC],
                            in_=v_sb[:])

            for buf_in, buf_out in ((kc[0], kg[0]), (kc[1], kg[1]),
                                    (vc[0], vg[0]), (vc[1], vg[1])):
                nc.gpsimd.collective_compute(
                    "AllGather", mybir.AluOpType.bypass,
                    replica_groups=groups,
                    ins=[buf_in.opt()], outs=[buf_out.opt()])

            # load gathered v into SBUF
            for kt in range(NKT):
                g, r = (0, kt) if kt < 8 else (1, kt - 8)
                nc.sync.dma_start(out=vt[kt][:], in_=vg[g][r])

        def kt_dram(kt):
            return kg[0][kt] if kt < 8 else kg[1][kt - 8]

        # ---- Phase D: attention + output projection ---------------------
        with tc.tile_pool(name="kts", bufs=2) as ktsp, \
             tc.tile_pool(name="mp", bufs=3) as mp, \
             tc.tile_pool(name="ep", bufs=2) as ep, \
             tc.tile_pool(name="erp", bufs=NKT) as erp, \
             tc.tile_pool(name="rp", bufs=2) as rp, \
             tc.tile_pool(name="wvt", bufs=1) as wvtp, \
             tc.tile_pool(name="wop", bufs=8) as wop, \
             tc.tile_pool(name="osb", bufs=2) as osb:

            m_off = 0
            for ci, nkt_c in enumerate(kt_depths):
                qp_mine = qpa if ci == 0 else qpb

                # pass 1: E_r[kt] = exp(SCALE * qkT) * M
                ers = []
                for kt in range(nkt_c):
                    kt_sb = ktsp.tile([P, D], F32R, name=f"kts{ci}_{kt}",
                                      tag="kts")
                    nc.sync.dma_start(out=kt_sb[:].rearrange(
                        "p (a f) -> p a f", a=DK),
                        in_=kt_dram(kt).transpose([1, 0, 2]))
                    acc = psum_acc.tile([P, SQC], F32, tag="acc")
                    for dk in range(DK):
                        nc.tensor.matmul(
                            out=acc[:], lhsT=kt_sb[:, dk * P:(dk + 1) * P],
                            rhs=qp_mine[dk][:], start=(dk == 0),
                            stop=(dk == DK - 1))
                    e_t = ep.tile([P, SQC], F32, name=f"e{ci}_{kt}", tag="e")
                    nc.scalar.activation(e_t[:], acc[:],
                                         mybir.ActivationFunctionType.Exp,
                                         bias=0.0, scale=SCALE)
                    m_t = mp.tile([P, SQC], F32, name=f"m{ci}_{kt}", tag="m")
                    nc.sync.dma_start(out=m_t[:], in_=M[m_off + kt])
                    er = erp.tile([P, SQC], F32R, name=f"er{ci}_{kt}",
                                  tag="er")
                    nc.vector.tensor_tensor(er[:], e_t[:], m_t[:],
                                            mybir.AluOpType.mult)
                    ers.append(er)
                m_off += nkt_c

                # denominator: den[s] = sum_t E_r[t, s]
                den = psum_den.tile([1, SQC], F32, tag="den")
                for kt in range(nkt_c):
                    nc.tensor.matmul(out=den[:], lhsT=ones_r[:],
                                     rhs=ers[kt][:], start=(kt == 0),
                                     stop=(kt == nkt_c - 1))
                recip = rp.tile([1, SQC], F32, name=f"recip{ci}", tag="recip")
                nc.vector.reciprocal(recip[:], den[:])
                recip_bc = rp.tile([P, SQC], F32, name=f"recipbc{ci}",
                                   tag="recipbc")
                nc.gpsimd.partition_broadcast(recip_bc[:], recip[:])

                # pass 2: wvT[dv, s] = (sum_t v[t, dv] E_r[t, s]) / den[s]
                wvts = [wvtp.tile([P, SQC], F32R, name=f"wvt{ci}_{d}",
                                  tag=f"wvt{d}") for d in range(DK)]
                for dh in range(2):
                    pvs = []
                    for dc in range(4):
                        pv = psum_pv.tile([P, SQC], F32, tag=f"pv{dc}")
                        dv = dh * 4 + dc
                        for kt in range(nkt_c):
                            nc.tensor.matmul(
                                out=pv[:],
                                lhsT=vt[kt][:, dv * P:(dv + 1) * P],
                                rhs=ers[kt][:], start=(kt == 0),
                                stop=(kt == nkt_c - 1))
                        pvs.append(pv)
                    for dc in range(4):
                        nc.vector.tensor_tensor(wvts[dh * 4 + dc][:],
                                                pvs[dc][:], recip_bc[:],
                                                mybir.AluOpType.mult)

                # pass 3: out rows = wvT^T @ Wo + bo
                for nch in range(2):
                    wos = [wop.tile([P, SQC], F32R, name=f"wo{ci}_{nch}_{d}",
                                    tag="wo") for d in range(DK)]
                    for dk in range(DK):
                        nc.sync.dma_start(
                            out=wos[dk][:],
                            in_=wo[dk * P:(dk + 1) * P,
                                   nch * SQC:(nch + 1) * SQC])
                    for st in range(4):
                        acc = psum_acc.tile([P, SQC], F32, tag="acc")
                        for dk in range(DK):
                            nc.tensor.matmul(
                                out=acc[:],
                                lhsT=wvts[dk][:, st * P:(st + 1) * P],
                                rhs=wos[dk][:], start=(dk == 0),
                                stop=(dk == DK - 1))
                        o_sb = osb.tile([P, SQC], F32,
                                        name=f"o{ci}_{nch}_{st}", tag="osb")
                        nc.vector.tensor_tensor(
                            o_sb[:], acc[:],
                            bo_bc[:, nch * SQC:(nch + 1) * SQC],
                            mybir.AluOpType.add)
                        nc.sync.dma_start(
                            out=out[ci * SQC + st * P:ci * SQC + (st + 1) * P,
                                    nch * SQC:(nch + 1) * SQC],
                            in_=o_sb[:])

    nc.compile()
    return nc


def _get_program(kt_depths):
    if kt_depths not in _PROGRAM_CACHE:
        _PROGRAM_CACHE[kt_depths] = _build_program(kt_depths)
    return _PROGRAM_CACHE[kt_depths]


# Row blocks (512 rows each) of the original sequence order
_BLK = [np.arange(i * 512, (i + 1) * 512) for i in range(4)]
# Gathered key order: AllGather#1 = [h0 chunkA rows, h1 chunkA rows],
# AllGather#2 = [h0 chunkB rows, h1 chunkB rows]
_KEY_ROWS = np.concatenate([_BLK[0], _BLK[1], _BLK[3], _BLK[2]])
# Per-half query rows: (chunk A rows, chunk B rows)
_MINE = ((_BLK[0], _BLK[3]), (_BLK[1], _BLK[2]))


def _is_causal(mask):
    i = np.arange(S)
    tri = np.where(i[:, None] >= i[None, :], np.float32(0.0),
                   np.float32(-1e9))
    return np.array_equal(mask, tri)


def _mask_tiles(emask, h, kt_depths):
    """M[c_kt, i, j] = exp(mask)[query_row(c, j), key_row(kt, i)]"""
    tiles = []
    for ci, nkt_c in enumerate(kt_depths):
        qrows = _MINE[h][ci]
        for kt in range(nkt_c):
            krows = _KEY_ROWS[kt * P:(kt + 1) * P]
            tiles.append(emask[np.ix_(qrows, krows)].T)
    return np.ascontiguousarray(np.stack(tiles))


def kernel(q, mask, Wq, bq, Wk, Wv, bv, Wo, bo):
    q = np.asarray(q, dtype=np.float32)
    mask = np.asarray(mask, dtype=np.float32)
    causal = _is_causal(mask)
    kt_depths = (8, 16) if causal else (16, 16)
    nc = _get_program(kt_depths)

    emask = np.exp(mask.astype(np.float64)).astype(np.float32)
    m_variants = [_mask_tiles(emask, h, kt_depths) for h in range(2)]

    in_maps = []
    for c in range(NCORES):
        b, h = c // 2, c % 2
        rows = np.concatenate(_MINE[h])
        qTp = np.ascontiguousarray(q[b][rows].T)
        in_maps.append({
            "qT": qTp, "M": m_variants[h],
            "Wq": Wq, "Wk": Wk, "Wv": Wv, "Wo": Wo,
            "bq": np.asarray(bq, np.float32),
            "bv": np.asarray(bv, np.float32),
            "bo": np.asarray(bo, np.float32),
        })

    res = run_bass_kernel_spmd(nc, in_maps, core_ids=list(range(NCORES)))

    out = np.empty((B, S, D), dtype=np.float32)
    for c in range(NCORES):
        b, h = c // 2, c % 2
        co = res.results[c]["out"]
        out[b, _MINE[h][0]] = co[0:SQC]
        out[b, _MINE[h][1]] = co[SQC:2 * SQC]
    return out


# revision 12
# speedup vs baseline: 1.0013x; 1.0013x over previous
"""Fused multi-head-attention-block kernel for 8 Trainium2 NeuronCores.

Reference computation (B=4, S=2048, D=1024):
    qp  = q @ Wq + bq
    k   = qp @ Wk
    v   = qp @ Wv + bv
    qk  = einsum('bsd,btd->bst', qp, k) * (D//16)**-0.25 + mask
    out = softmax(qk) @ v @ Wo + bo

Sharding: each core owns one (batch, half-of-queries) pair.  The two cores
of a batch split the projection phases (each computes qpT/kT/v for its own
1024 query rows only), then exchange kT and v via pair-wise AllGathers.
Attention + output projection run on each core's own rows.

All activations are kept transposed ([dim, seq]) so every matmul's
contraction dim lands on SBUF partitions with zero on-chip transposes:
    qpT = Wq^T @ qT          kT = Wk^T @ qpT        v = qpT^T @ Wv
    qkT[t,s] = kT^T@qpT      wvT = v^T @ E          out = wvT^T @ Wo
Softmax runs over the PARTITION dim of qkT: exp on ScalarE (scale fused,
no max-subtraction -- logits are bounded and exp(-1e9) -> 0 exactly on the
ACT LUT), multiplicative mask tiles M = exp(mask) from the host (handles
causal and arbitrary additive masks uniformly), denominator via a
ones-vector matmul accumulated across key tiles, division folded into the
PSUM eviction of the PV matmul.

Key-tile order after the gathers is rows [0:512 | 512:1024 | 1536:2048 |
1024:1536] (AllGather #1 carries both cores' chunk-A rows, #2 the chunk-B
rows).  With a causal mask, chunk A (query positions 0:512) only needs the
first 8 key tiles (= rows 0:1024 cover every core's chunk-A queries), so
the per-core loop structure is {8,16} key tiles -- identical on all cores
(SPMD), 25% fewer attention matmuls.  A host-side query split per half
makes this work: h=0 owns rows {0:512, 1536:2048}, h=1 owns rows
{512:1024, 1024:1536}; masks are data, so any mask is supported (dense
fallback {16,16}).

Matmuls run in float32r (TF32-like, 4x fp32 throughput, ~2e-4 rel err).
"""

import os
import sys
from contextlib import ExitStack

for _p in ("/opt/trn_rl_repo",):
    if _p not in sys.path and os.path.isdir(_p):
        sys.path.append(_p)

import numpy as np

import concourse.bass as bass
import concourse.mybir as mybir
import concourse.tile as tile
from concourse import bacc
from concourse.bass_utils import run_bass_kernel_spmd

B, S, D, N_HEAD = 4, 2048, 1024, 16
P = 128                      # SBUF partitions
NCORES = 8
SQC = 512                    # query-chunk width (free dim of every matmul)
DK = D // P                  # 8 contraction tiles
NKT = S // P                 # 16 key tiles
SCALE = float((D // N_HEAD) ** (-0.25))
F32 = mybir.dt.float32
F32R = mybir.dt.float32r

_PROGRAM_CACHE = {}


def _build_program(kt_depths):
    """Emit the SPMD Bass program.  kt_depths = (ktA, ktB): number of
    128-row key tiles processed for query chunk A (positions 0:512) and
    chunk B (positions 1536:2048)."""
    nc = bacc.Bacc("TRN2", target_bir_lowering=False, debug=False,
                   num_devices=NCORES)

    qT = nc.dram_tensor("qT", [D, 2 * SQC], F32R, kind="ExternalInput").ap()
    M = nc.dram_tensor("M", [sum(kt_depths), P, SQC], F32,
                       kind="ExternalInput").ap()
    wq = nc.dram_tensor("Wq", [D, D], F32R, kind="ExternalInput").ap()
    wk = nc.dram_tensor("Wk", [D, D], F32R, kind="ExternalInput").ap()
    wv = nc.dram_tensor("Wv", [D, D], F32R, kind="ExternalInput").ap()
    wo = nc.dram_tensor("Wo", [D, D], F32R, kind="ExternalInput").ap()
    bq = nc.dram_tensor("bq", [D], F32, kind="ExternalInput").ap()
    bv = nc.dram_tensor("bv", [D], F32, kind="ExternalInput").ap()
    bo = nc.dram_tensor("bo", [D], F32, kind="ExternalInput").ap()
    out = nc.dram_tensor("out", [2 * SQC, D], F32, kind="ExternalOutput").ap()

    groups = [[0, 1], [2, 3], [4, 5], [6, 7]]

    with tile.TileContext(nc) as tc, ExitStack() as stack:
        const = stack.enter_context(tc.tile_pool(name="const", bufs=1))
        ones_f = const.tile([P, 1], F32)
        nc.gpsimd.memset(ones_f[:], 1.0)
        ones_r = const.tile([P, 1], F32R)
        nc.vector.tensor_copy(ones_r[:], ones_f[:])
        bq_sb = const.tile([P, DK], F32)
        nc.sync.dma_start(out=bq_sb[:], in_=bq.rearrange("(a p) -> p a", p=P))
        bv_row = const.tile([1, D], F32)
        nc.sync.dma_start(out=bv_row[:], in_=bv.unsqueeze(0))
        bo_row = const.tile([1, D], F32)
        nc.sync.dma_start(out=bo_row[:], in_=bo.unsqueeze(0))
        bv_bc = const.tile([P, D], F32)
        nc.gpsimd.partition_broadcast(bv_bc[:], bv_row[:])
        bo_bc = const.tile([P, D], F32)
        nc.gpsimd.partition_broadcast(bo_bc[:], bo_row[:])

        # persistent activation storage
        qpa_p = stack.enter_context(tc.tile_pool(name="qpa", bufs=1))
        qpb_p = stack.enter_context(tc.tile_pool(name="qpb", bufs=1))
        v_p = stack.enter_context(tc.tile_pool(name="vp", bufs=1))
        qpa = [qpa_p.tile([P, SQC], F32R, name=f"qpa{d}") for d in range(DK)]
        qpb = [qpb_p.tile([P, SQC], F32R, name=f"qpb{d}") for d in range(DK)]
        vt = [v_p.tile([P, D], F32R, name=f"v{t}") for t in range(NKT)]

        psum_acc = stack.enter_context(
            tc.tile_pool(name="ps_acc", bufs=2, space="PSUM"))
        psum_den = stack.enter_context(
            tc.tile_pool(name="ps_den", bufs=1, space="PSUM"))
        psum_pv = stack.enter_context(
            tc.tile_pool(name="ps_pv", bufs=1, space="PSUM"))

        dram = stack.enter_context(tc.tile_pool(name="dram", bufs=1,
                                                space="DRAM"))
        # kT contributions/gathers in [kt, dk, 128, 128] tile layout
        kc = [dram.tile([NKT // 4, DK, P, P], F32R, name=f"kc{c}")
              for c in range(2)]
        kg = [dram.tile([NKT // 2, DK, P, P], F32R, name=f"kg{c}")
              for c in range(2)]
        vc = [dram.tile([NKT // 4, P, D], F32R, name=f"vc{c}")
              for c in range(2)]
        vg = [dram.tile([NKT // 2, P, D], F32R, name=f"vg{c}")
              for c in range(2)]

        # ---- Phases A-C: projections for this core's 1024 rows ----------
        with tc.tile_pool(name="wpool", bufs=9) as wp, \
             tc.tile_pool(name="qtpool", bufs=16) as qtp, \
             tc.tile_pool(name="kev", bufs=3) as kev, \
             tc.tile_pool(name="vev", bufs=3) as vev:

            w_q = [wp.tile([P, D], F32R, name=f"wq{d}", tag="w")
                   for d in range(DK)]
            qts = [[None] * DK for _ in range(2)]
            # interleave weight/activation loads so the first matmul's
            # inputs arrive early
            for dk in range(DK):
                nc.sync.dma_start(out=w_q[dk][:],
                                  in_=wq[dk * P:(dk + 1) * P, :])
                t = qtp.tile([P, SQC], F32R, name=f"qt0_{dk}", tag="qt")
                nc.sync.dma_start(out=t[:],
                                  in_=qT[dk * P:(dk + 1) * P, 0:SQC])
                qts[0][dk] = t
            for dk in range(DK):
                t = qtp.tile([P, SQC], F32R, name=f"qt1_{dk}", tag="qt")
                nc.sync.dma_start(out=t[:],
                                  in_=qT[dk * P:(dk + 1) * P, SQC:2 * SQC])
                qts[1][dk] = t

            def gather(buf_in, buf_out):
                nc.gpsimd.collective_compute(
                    "AllGather", mybir.AluOpType.bypass,
                    replica_groups=groups,
                    ins=[buf_in.opt()], outs=[buf_out.opt()])

            def phase_a(ch, qp_dst):
                # qpT for query chunk ch
                for do in range(DK):
                    acc = psum_acc.tile([P, SQC], F32, tag="acc")
                    for dk in range(DK):
                        nc.tensor.matmul(
                            out=acc[:], lhsT=w_q[dk][:, do * P:(do + 1) * P],
                            rhs=qts[ch][dk][:], start=(dk == 0),
                            stop=(dk == DK - 1))
                    nc.vector.tensor_tensor(
                        qp_dst[do][:], acc[:],
                        bq_sb[:, do:do + 1].to_broadcast([P, SQC]),
                        mybir.AluOpType.add)

            def phase_b(ch, qp_src):
                # kT for chunk ch -> contribution buffer
                for do in range(DK):
                    acc = psum_acc.tile([P, SQC], F32, tag="acc")
                    for dk in range(DK):
                        nc.tensor.matmul(
                            out=acc[:], lhsT=w_k[dk][:, do * P:(do + 1) * P],
                            rhs=qp_src[dk][:], start=(dk == 0),
                            stop=(dk == DK - 1))
                    kt_sb = kev.tile([P, SQC], F32R, name=f"kev{ch}_{do}",
                                     tag="kev")
                    nc.vector.tensor_copy(kt_sb[:], acc[:])
                    nc.sync.dma_start(
                        out=kc[ch][0:4, do].transpose([1, 0, 2]),
                        in_=kt_sb[:].rearrange("p (a f) -> p a f", a=4))

            def phase_c(ch, qp_src, w_v):
                # v for chunk ch -> contribution buffer
                for st in range(4):
                    for nch in range(2):
                        acc = psum_acc.tile([P, SQC], F32, tag="acc")
                        for dk in range(DK):
                            nc.tensor.matmul(
                                out=acc[:],
                                lhsT=qp_src[dk][:, st * P:(st + 1) * P],
                                rhs=w_v[dk][:, nch * SQC:(nch + 1) * SQC],
                                start=(dk == 0), stop=(dk == DK - 1))
                        v_sb = vev.tile([P, SQC], F32R,
                                        name=f"vev{ch}_{st}_{nch}", tag="vev")
                        nc.vector.tensor_tensor(
                            v_sb[:], acc[:],
                            bv_bc[:, nch * SQC:(nch + 1) * SQC],
                            mybir.AluOpType.add)
                        nc.sync.dma_start(
                            out=vc[ch][st, :, nch * SQC:(nch + 1) * SQC],
                            in_=v_sb[:])

            # Phase order puts each AllGather as early as possible so the
            # gathers hide behind the remaining projection matmuls (the PE
            # stream is in-order, collectives run on the CC queue).
            phase_a(0, qpa)
            phase_a(1, qpb)
            w_k = [wp.tile([P, D], F32R, name=f"wk{d}", tag="w")
                   for d in range(DK)]
            for d in range(DK):
                nc.sync.dma_start(out=w_k[d][:], in_=wk[d * P:(d + 1) * P, :])
            phase_b(0, qpa)
            gather(kc[0], kg[0])
            phase_b(1, qpb)
            gather(kc[1], kg[1])
            w_v = [wp.tile([P, D], F32R, name=f"wv{d}", tag="w")
                   for d in range(DK)]
            for d in range(DK):
                nc.sync.dma_start(out=w_v[d][:], in_=wv[d * P:(d + 1) * P, :])
            phase_c(0, qpa, w_v)
            gather(vc[0], vg[0])
            phase_c(1, qpb, w_v)
            gather(vc[1], vg[1])

            # load gathered v into SBUF
            for kt in range(NKT):
                g, r = (0, kt) if kt < 8 else (1, kt - 8)
                nc.sync.dma_start(out=vt[kt][:], in_=vg[g][r])

        def kt_dram(kt):
            return kg[0][kt] if kt < 8 else kg[1][kt - 8]

        # ---- Phase D: attention + output projection ---------------------
        with tc.tile_pool(name="kts", bufs=2) as ktsp, \
             tc.tile_pool(name="mp", bufs=3) as mp, \
             tc.tile_pool(name="ep", bufs=2) as ep, \
             tc.tile_pool(name="erp", bufs=NKT) as erp, \
             tc.tile_pool(name="rp", bufs=2) as rp, \
             tc.tile_pool(name="wvt", bufs=1) as wvtp, \
             tc.tile_pool(name="wop", bufs=8) as wop, \
             tc.tile_pool(name="osb", bufs=2) as osb:

            m_off = 0
            for ci, nkt_c in enumerate(kt_depths):
                qp_mine = qpa if ci == 0 else qpb

                # pass 1: E_r[kt] = exp(SCALE * qkT) * M
                ers = []
                for kt in range(nkt_c):
                    kt_sb = ktsp.tile([P, D], F32R, name=f"kts{ci}_{kt}",
                                      tag="kts")
                    nc.sync.dma_start(out=kt_sb[:].rearrange(
                        "p (a f) -> p a f", a=DK),
                        in_=kt_dram(kt).transpose([1, 0, 2]))
                    acc = psum_acc.tile([P, SQC], F32, tag="acc")
                    for dk in range(DK):
                        nc.tensor.matmul(
                            out=acc[:], lhsT=kt_sb[:, dk * P:(dk + 1) * P],
                            rhs=qp_mine[dk][:], start=(dk == 0),
                            stop=(dk == DK - 1))
                    e_t = ep.tile([P, SQC], F32, name=f"e{ci}_{kt}", tag="e")
                    nc.scalar.activation(e_t[:], acc[:],
                                         mybir.ActivationFunctionType.Exp,
                                         bias=0.0, scale=SCALE)
                    m_t = mp.tile([P, SQC], F32, name=f"m{ci}_{kt}", tag="m")
                    nc.sync.dma_start(out=m_t[:], in_=M[m_off + kt])
                    er = erp.tile([P, SQC], F32R, name=f"er{ci}_{kt}",
                                  tag="er")
                    nc.vector.tensor_tensor(er[:], e_t[:], m_t[:],
                                            mybir.AluOpType.mult)
                    ers.append(er)
                m_off += nkt_c

                # denominator: den[s] = sum_t E_r[t, s]
                den = psum_den.tile([1, SQC], F32, tag="den")
                for kt in range(nkt_c):
                    nc.tensor.matmul(out=den[:], lhsT=ones_r[:],
                                     rhs=ers[kt][:], start=(kt == 0),
                                     stop=(kt == nkt_c - 1))
                recip = rp.tile([1, SQC], F32, name=f"recip{ci}", tag="recip")
                nc.vector.reciprocal(recip[:], den[:])
                recip_bc = rp.tile([P, SQC], F32, name=f"recipbc{ci}",
                                   tag="recipbc")
                nc.gpsimd.partition_broadcast(recip_bc[:], recip[:])

                # pass 2: wvT[dv, s] = (sum_t v[t, dv] E_r[t, s]) / den[s]
                wvts = [wvtp.tile([P, SQC], F32R, name=f"wvt{ci}_{d}",
                                  tag=f"wvt{d}") for d in range(DK)]
                for dh in range(2):
                    pvs = []
                    for dc in range(4):
                        pv = psum_pv.tile([P, SQC], F32, tag=f"pv{dc}")
                        dv = dh * 4 + dc
                        for kt in range(nkt_c):
                            nc.tensor.matmul(
                                out=pv[:],
                                lhsT=vt[kt][:, dv * P:(dv + 1) * P],
                                rhs=ers[kt][:], start=(kt == 0),
                                stop=(kt == nkt_c - 1))
                        pvs.append(pv)
                    for dc in range(4):
                        nc.vector.tensor_tensor(wvts[dh * 4 + dc][:],
                                                pvs[dc][:], recip_bc[:],
                                                mybir.AluOpType.mult)

                # pass 3: out rows = wvT^T @ Wo + bo
                for nch in range(2):
                    wos = [wop.tile([P, SQC], F32R, name=f"wo{ci}_{nch}_{d}",
                                    tag="wo") for d in range(DK)]
                    for dk in range(DK):
                        nc.sync.dma_start(
                            out=wos[dk][:],
                            in_=wo[dk * P:(dk + 1) * P,
                                   nch * SQC:(nch + 1) * SQC])
                    for st in range(4):
                        acc = psum_acc.tile([P, SQC], F32, tag="acc")
                        for dk in range(DK):
                            nc.tensor.matmul(
                                out=acc[:],
                                lhsT=wvts[dk][:, st * P:(st + 1) * P],
                                rhs=wos[dk][:], start=(dk == 0),
                                stop=(dk == DK - 1))
                        o_sb = osb.tile([P, SQC], F32,
                                        name=f"o{ci}_{nch}_{st}", tag="osb")
                        nc.vector.tensor_tensor(
                            o_sb[:], acc[:],
                            bo_bc[:, nch * SQC:(nch + 1) * SQC],
                            mybir.AluOpType.add)
                        nc.sync.dma_start(
                            out=out[ci * SQC + st * P:ci * SQC + (st + 1) * P,
                                    nch * SQC:(nch + 1) * SQC],
                            in_=o_sb[:])

    nc.compile()
    return nc


def _get_program(kt_depths):
    if kt_depths not in _PROGRAM_CACHE:
        _PROGRAM_CACHE[kt_depths] = _build_program(kt_depths)
    return _PROGRAM_CACHE[kt_depths]


# Row blocks (512 rows each) of the original sequence order
_BLK = [np.arange(i * 512, (i + 1) * 512) for i in range(4)]
# Gathered key order: AllGather#1 = [h0 chunkA rows, h1 chunkA rows],
# AllGather#2 = [h0 chunkB rows, h1 chunkB rows]
_KEY_ROWS = np.concatenate([_BLK[0], _BLK[1], _BLK[3], _BLK[2]])
# Per-half query rows: (chunk A rows, chunk B rows)
_MINE = ((_BLK[0], _BLK[3]), (_BLK[1], _BLK[2]))


def _is_causal(mask):
    i = np.arange(S)
    tri = np.where(i[:, None] >= i[None, :], np.float32(0.0),
                   np.float32(-1e9))
    return np.array_equal(mask, tri)


def _mask_tiles(emask, h, kt_depths):
    """M[c_kt, i, j] = exp(mask)[query_row(c, j), key_row(kt, i)]"""
    tiles = []
    for ci, nkt_c in enumerate(kt_depths):
        qrows = _MINE[h][ci]
        for kt in range(nkt_c):
            krows = _KEY_ROWS[kt * P:(kt + 1) * P]
            tiles.append(emask[np.ix_(qrows, krows)].T)
    return np.ascontiguousarray(np.stack(tiles))


def kernel(q, mask, Wq, bq, Wk, Wv, bv, Wo, bo):
    q = np.asarray(q, dtype=np.float32)
    mask = np.asarray(mask, dtype=np.float32)
    causal = _is_causal(mask)
    kt_depths = (8, 16) if causal else (16, 16)
    nc = _get_program(kt_depths)

    emask = np.exp(mask.astype(np.float64)).astype(np.float32)
    m_variants = [_mask_tiles(emask, h, kt_depths) for h in range(2)]

    in_maps = []
    for c in range(NCORES):
        b, h = c // 2, c % 2
        rows = np.concatenate(_MINE[h])
        qTp = np.ascontiguousarray(q[b][rows].T)
        in_maps.append({
            "qT": qTp, "M": m_variants[h],
            "Wq": Wq, "Wk": Wk, "Wv": Wv, "Wo": Wo,
            "bq": np.asarray(bq, np.float32),
            "bv": np.asarray(bv, np.float32),
            "bo": np.asarray(bo, np.float32),
        })

    res = run_bass_kernel_spmd(nc, in_maps, core_ids=list(range(NCORES)))

    out = np.empty((B, S, D), dtype=np.float32)
    for c in range(NCORES):
        b, h = c // 2, c % 2
        co = res.results[c]["out"]
        out[b, _MINE[h][0]] = co[0:SQC]
        out[b, _MINE[h][1]] = co[SQC:2 * SQC]
    return out
